# revision 8
# baseline (speedup 1.0000x reference)
"""ChebConv GNN (3 layers, K=4) on 8 Trainium2 NeuronCores.

Pull-mode graph-parallel SpMM: nodes are partitioned across the 8 cores
(LPT on in-degree into 400 windows of 128 dst nodes). Each SpMM gathers
source rows from a replicated fp16 node-major table in HBM (SWDGE
dma_gather, persistent SBUF-resident indices, 4 queues), segment-sums
them per 128-dst window with a BINARY one-hot matmul on the TensorEngine
(the ChebConv edge weights -dinv[s]*dinv[d] are factorized: dinv[s] is
pre-folded into the table values, dinv[d] applied as a per-window
post-scale), and per-core slices are re-replicated with half-table
AllGather chunks, the first fired mid-SpMM to overlap the collective
with compute.

Compute layout is feature-major ([feature, node]); node-major table
slices are produced with PE transposes, the dinv prescale fused into the
PSUM-drain copy. All tables/messages/weights are fp16 with fp32 PSUM
accumulation.
"""

import numpy as np

# ---------------- problem constants (hardcoded per contract) ----------------
N, E = 50000, 800000
F, HID, CLS, K = 128, 128, 40, 4
P = 128
CORES = 8
NW = 50                 # dst windows per core
SL = NW * P             # 6400 nodes per core
NPAD = CORES * SL       # 51200 padded node count
NPAIR = NW // 2         # window pairs per core
WSPLIT = 32             # windows 0..31 -> half A (64%), rest -> half B
HALFA = CORES * WSPLIT * P      # 30720 rows (< 32768, int16-safe)
HALFB = NPAD - HALFA            # 20480 rows
HROWSA = WSPLIT * P             # 3840 slice rows (half A, per core)
HROWSB = (NW - WSPLIT) * P      # 2560 slice rows (half B, per core)
GDELAY = 5              # pairs of A-gather lookahead past B-gathers


# ---------------- host preprocessing ----------------
def _lpt_windows(indeg, n_windows, cap):
    """Assign nodes to windows (cap nodes each), balancing in-degree sums.
    Returns perm: old node id -> new node id."""
    import heapq
    order = np.argsort(-indeg, kind="stable")
    heap = [(0, wi) for wi in range(n_windows)]
    heapq.heapify(heap)
    counts = np.zeros(n_windows, np.int64)
    perm = np.empty(len(indeg), np.int64)
    for old in order:
        while True:
            load, wi = heapq.heappop(heap)
            if counts[wi] < cap:
                break
        perm[old] = wi * cap + counts[wi]
        counts[wi] += 1
        if counts[wi] < cap:
            heapq.heappush(heap, (load + int(indeg[old]), wi))
    return perm


def _table_row(nid):
    """New node id -> row in the half-major table layout:
    [half h][core c][local window][pos 128]."""
    c = nid // SL
    w = (nid % SL) // P
    p = nid % P
    h = (w >= WSPLIT).astype(np.int64) if hasattr(w, 'astype') else int(w >= WSPLIT)
    wl = w - h * WSPLIT
    hrows = np.where(h, HROWSB, HROWSA) if hasattr(w, 'astype') else (HROWSB if h else HROWSA)
    return h * HALFA + c * hrows + wl * P + p


def _preprocess(edge_src, edge_dst, n):
    es = np.asarray(edge_src, np.int64)
    ed = np.asarray(edge_dst, np.int64)
    deg = np.bincount(es, minlength=n).astype(np.float32)
    dinv = np.where(deg > 0, 1.0 / np.sqrt(np.maximum(deg, 1.0)), 0.0).astype(
        np.float32
    )

    indeg = np.bincount(ed, minlength=n)
    perm = _lpt_windows(indeg, CORES * NW, P)  # old -> new

    dinv_new = np.zeros(NPAD, np.float32)
    dinv_new[perm] = dinv

    nsrc = perm[es]
    ndst = perm[ed]
    core_e = ndst // SL
    win_e = (ndst % SL) // P
    dloc_e = (ndst % P).astype(np.int64)
    srow = _table_row(nsrc)
    half_e = (srow >= HALFA).astype(np.int64)
    lrow_e = srow - half_e * HALFA  # local row within half table (int16-safe)

    # group edges by (core, win, half)
    gkey = (core_e * NW + win_e) * 2 + half_e
    ngroups = CORES * NW * 2
    order = np.argsort(gkey, kind="stable")
    gkey_s = gkey[order]
    counts = np.bincount(gkey_s, minlength=ngroups)
    starts = np.concatenate([[0], np.cumsum(counts)[:-1]])
    rank = np.arange(len(es)) - starts[gkey_s]

    cnts = counts.reshape(CORES, NW, 2)
    CA = max(int(np.ceil(cnts[:, :, 0].max() / P)), 1)
    CB = max(int(np.ceil(cnts[:, :, 1].max() / P)), 1)
    CW = CA + CB

    capa = {0: CA * P, 1: CB * P}
    idx_pad = {h: np.zeros((CORES, NW, capa[h]), np.int16) for h in (0, 1)}
    dl_pad = np.full((CORES, NW, CW, P), 255.0, np.float32)

    ce, we, he = core_e[order], win_e[order], half_e[order]
    de, ie = dloc_e[order], lrow_e[order]
    for h in (0, 1):
        m = he == h
        idx_pad[h][ce[m], we[m], rank[m]] = ie[m].astype(np.int16)
        coff = rank[m] // P + (0 if h == 0 else CA)
        dl_pad[ce[m], we[m], coff, rank[m] % P] = de[m]

    # persistent wrapped idx arrays: per pair g, A block = [win 2g cols,
    # win 2g+1 cols] (2*CA*128 idxs), concatenated over the 25 pairs.
    def wrap(idxs):  # [cores, npair, L] -> [cores, 128, npair*(L//16)]
        c, g, L = idxs.shape
        a = idxs.reshape(c, g, L // 16, 16).transpose(0, 1, 3, 2)
        a = np.tile(a, (1, 1, 8, 1))  # [c, g, 128, L//16]
        return np.concatenate([a[:, i] for i in range(g)], axis=2).copy()

    idxA = wrap(idx_pad[0].reshape(CORES, NPAIR, 2 * CA * P))
    idxB = wrap(idx_pad[1].reshape(CORES, NPAIR, 2 * CB * P))

    dl_arr = dl_pad.transpose(0, 3, 1, 2).reshape(CORES, P, NW * CW).copy()

    return dict(perm=perm, dinv_new=dinv_new, CA=CA, CB=CB, CW=CW,
                idxA=idxA, idxB=idxB, dl=dl_arr)


# ---------------- device kernel ----------------
def _build(CA, CB, stage=99):
    import concourse.bass as bass
    import concourse.bacc as bacc
    import concourse.tile as tile
    import concourse.mybir as mybir
    import dataclasses

    CW = CA + CB
    fp = mybir.dt.float32
    f16 = mybir.dt.float16
    Alu = mybir.AluOpType
    Act = mybir.ActivationFunctionType

    nc = bacc.Bacc("TRN2", target_bir_lowering=False, debug=False,
                   num_devices=CORES, num_swdge_queues=4)

    # -------- I/O --------
    xT_d = nc.dram_tensor("xT", [P, SL], f16, kind="ExternalInput")
    xfull_d = nc.dram_tensor("xfull", [NPAD, F], f16, kind="ExternalInput")
    ixA_d = nc.dram_tensor("ixA", [P, NPAIR * 2 * CA * 8], mybir.dt.int16,
                           kind="ExternalInput")
    ixB_d = nc.dram_tensor("ixB", [P, NPAIR * 2 * CB * 8], mybir.dt.int16,
                           kind="ExternalInput")
    dl_d = nc.dram_tensor("dl", [P, NW * CW], f16, kind="ExternalInput")
    d1_d = nc.dram_tensor("d1", [P, SL], f16, kind="ExternalInput")
    d2_d = nc.dram_tensor("d2", [P, SL], f16, kind="ExternalInput")
    dinvP_d = nc.dram_tensor("dinvP", [P, NW], fp, kind="ExternalInput")
    w0_d = nc.dram_tensor("w0t", [P, K, HID], f16, kind="ExternalInput")
    w1_d = nc.dram_tensor("w1t", [P, K, HID], f16, kind="ExternalInput")
    w2_d = nc.dram_tensor("w2t", [P, K, CLS], f16, kind="ExternalInput")
    b0_d = nc.dram_tensor("b0", [HID, 1], fp, kind="ExternalInput")
    b1_d = nc.dram_tensor("b1", [HID, 1], fp, kind="ExternalInput")
    b2_d = nc.dram_tensor("b2", [CLS, 1], fp, kind="ExternalInput")
    iota_d = nc.dram_tensor("iota", [P, P], f16, kind="ExternalInput")
    ident_d = nc.dram_tensor("ident", [P, P], f16, kind="ExternalInput")
    out_d = nc.dram_tensor("out", [SL, CLS], fp, kind="ExternalOutput")

    def bmid(ap, n):  # [128, X] -> [128, n, X], middle stride 0
        return dataclasses.replace(ap, ap=[ap.ap[0], [0, n], ap.ap[1]])

    def blast(ap, n):  # [128, X] -> [128, X, n], last stride 0
        return dataclasses.replace(ap, ap=[ap.ap[0], ap.ap[1], [0, n]])

    nA, nB = 2 * CA * P, 2 * CB * P

    with tile.TileContext(nc) as tc:
        with (
            tc.tile_pool(name="const", bufs=1) as constp,
            tc.tile_pool(name="tx", bufs=3) as txp,
            tc.tile_pool(name="acc", bufs=1) as accp,
            tc.tile_pool(name="g", bufs=7) as gp,
            tc.tile_pool(name="m", bufs=2) as mp,
            tc.tile_pool(name="tmp", bufs=3) as tmpp,
            tc.tile_pool(name="st", bufs=6) as stp,
            tc.tile_pool(name="psA", bufs=3, space="PSUM") as psA,
            tc.tile_pool(name="psT", bufs=3, space="PSUM") as psT,
            tc.tile_pool(name="psW", bufs=2, space="PSUM") as psW,
            tc.tile_pool(name="dram", bufs=4, space="DRAM") as dramp,
            tc.tile_pool(name="tabs", bufs=4, space="DRAM") as tabp,
        ):
            # -------- constants --------
            ixA_t = constp.tile([P, NPAIR * 2 * CA * 8], mybir.dt.int16)
            ixB_t = constp.tile([P, NPAIR * 2 * CB * 8], mybir.dt.int16)
            dl_t = constp.tile([P, NW * CW], f16)
            d1_t = constp.tile([P, SL], f16)
            d2_t = constp.tile([P, SL], f16)
            dinvP_t = constp.tile([P, NW], fp)
            iota_t = constp.tile([P, P], f16)
            ident_t = constp.tile([P, P], f16)
            w0_t = constp.tile([P, K, HID], f16)
            w1_t = constp.tile([P, K, HID], f16)
            w2_t = constp.tile([P, K, CLS], f16)
            b0_t = constp.tile([HID, 1], fp)
            b1_t = constp.tile([HID, 1], fp)
            b2_t = constp.tile([CLS, 1], fp)
            for t, d in ((ixA_t, ixA_d), (ixB_t, ixB_d), (dl_t, dl_d),
                         (d1_t, d1_d), (d2_t, d2_d), (dinvP_t, dinvP_d),
                         (iota_t, iota_d), (ident_t, ident_d),
                         (w0_t, w0_d), (w1_t, w1_d), (w2_t, w2_d),
                         (b0_t, b0_d), (b1_t, b1_d), (b2_t, b2_d)):
                nc.sync.dma_start(out=t[:], in_=d[:])

            tx0 = txp.tile([P, SL], f16, tag="tx")
            nc.sync.dma_start(out=tx0[:], in_=xT_d[:, :])

            tabA_in = xfull_d[0:HALFA, :]
            tabB_in = xfull_d[HALFA:NPAD, :]

            def ag_half(slice_h, h):
                """AllGather one half-slice into a fresh half-table."""
                tab = tabp.tile([HALFA if h == 0 else HALFB, F], f16,
                                tag="tab", addr_space="Shared",
                                name=f"tab{h}")
                nc.gpsimd.collective_compute(
                    "AllGather", Alu.bypass,
                    replica_groups=[list(range(CORES))],
                    ins=[slice_h[:, :].opt()],
                    outs=[tab[:, :].opt()])
                return tab

            def store_win(src_fm, w, slices):
                """Transpose window w of feature-major src, scale by dinv,
                store node-major into the half slice tile."""
                wb = slice(w * P, (w + 1) * P)
                pst = psT.tile([P, P], f16, tag="pst")
                nc.tensor.transpose(out=pst[:], in_=src_fm[:, wb],
                                    identity=ident_t[:])
                st = stp.tile([P, P], f16, tag="st")
                nc.scalar.activation(out=st[:], in_=pst[:], func=Act.Copy,
                                     scale=dinvP_t[:, w:w + 1])
                h = int(w >= WSPLIT)
                wl = w - h * WSPLIT
                nc.sync.dma_start(out=slices[h][wl * P:(wl + 1) * P, :],
                                  in_=st[:])

            def spmm(tabA, tabB, tx_prev2, Wt, b_t, fo, acc, k,
                     want_slice, produce_h=None, produce_out=False):
                """One lhat hop (k-th Chebyshev term).
                want_slice: make the dinv-scaled table of tx_new + AG.
                produce_h: (hT, slicesH, tabH) -> relu(acc) table (k=3 hop
                of non-last layers).  produce_out: last-layer epilogue."""
                tx_new = txp.tile([P, SL], f16, tag="tx")
                slices = None
                tabs = [None, None]
                if want_slice:
                    slices = [dramp.tile([HROWSA if hh == 0 else HROWSB, F],
                                         f16, tag="sl", name=f"sl{hh}")
                              for hh in (0, 1)]
                Gs = {}

                def emit_A(g):
                    G = gp.tile([P, 2 * CW, P], f16, tag="G", name="G")
                    Gs[g] = G
                    nc.gpsimd.dma_gather(
                        out_ap=G[:, 0:2 * CA, :], in_ap=tabA,
                        idxs_ap=ixA_t[:, g * 2 * CA * 8:(g + 1) * 2 * CA * 8],
                        num_idxs=nA, num_idxs_reg=nA, elem_size=P,
                        single_packet=False, queue_num=(2 * g) % 4)

                for g in range(GDELAY):
                    emit_A(g)
                for g in range(NPAIR):
                    G = Gs.pop(g)
                    nc.gpsimd.dma_gather(
                        out_ap=G[:, 2 * CA:2 * CW, :], in_ap=tabB,
                        idxs_ap=ixB_t[:, g * 2 * CB * 8:(g + 1) * 2 * CB * 8],
                        num_idxs=nB, num_idxs_reg=nB, elem_size=P,
                        single_packet=False, queue_num=(2 * g + 1) % 4)
                    if g + GDELAY < NPAIR:
                        emit_A(g + GDELAY)
                    M2 = mp.tile([P, 2 * CW, P], f16, tag="M")
                    nc.vector.tensor_tensor(
                        out=M2[:], in0=bmid(iota_t[:], 2 * CW),
                        in1=blast(dl_t[:, 2 * g * CW:(2 * g + 2) * CW], P),
                        op=Alu.is_equal)
                    pss = []
                    for h in (0, 1):
                        M = M2[:, h * CW:(h + 1) * CW, :]
                        ps = psA.tile([P, P], fp, tag="ps")
                        pss.append(ps)
                        for c in range(CW):
                            Gsl = (G[:, h * CA + c, :] if c < CA
                                   else G[:, 2 * CA + h * CB + (c - CA), :])
                            nc.tensor.matmul(out=ps[:], lhsT=Gsl,
                                             rhs=M[:, c, :],
                                             start=(c == 0), stop=(c == CW - 1))
                    for h in (0, 1):
                        w = 2 * g + h
                        wb = slice(w * P, (w + 1) * P)
                        ps = pss[h]
                        if tx_prev2 is None:
                            nc.vector.tensor_tensor(
                                out=tx_new[:, wb], in0=ps[:],
                                in1=d1_t[:, wb], op=Alu.mult)
                        else:
                            tm = tmpp.tile([P, P], fp, tag="tm")
                            nc.vector.tensor_tensor(
                                out=tm[:], in0=ps[:], in1=d2_t[:, wb],
                                op=Alu.mult)
                            nc.vector.tensor_tensor(
                                out=tx_new[:, wb], in0=tm[:],
                                in1=tx_prev2[:, wb], op=Alu.subtract)
                    if want_slice:
                        for h in (0, 1):
                            store_win(tx_new, 2 * g + h, slices)
                    # ---- per-pair: W-matmul chunk, acc update, finales ----
                    ch = slice(2 * g * P, (2 * g + 2) * P)
                    pw = psW.tile([P, 2 * P], fp, tag="pw")
                    nc.tensor.matmul(out=pw[:fo, :], lhsT=Wt[:, k, :fo],
                                     rhs=tx_new[:, ch], start=True, stop=True)
                    nc.vector.tensor_tensor(out=acc[:fo, ch],
                                            in0=acc[:fo, ch],
                                            in1=pw[:fo, :], op=Alu.add)
                    if produce_h is not None:
                        hT, slicesH, tabsH = produce_h
                        nc.scalar.activation(out=hT[:, ch], in_=acc[:, ch],
                                             func=Act.Relu)
                        for h in (0, 1):
                            store_win(hT, 2 * g + h, slicesH)
                        if g == WSPLIT // 2 - 1:
                            tabsH[0] = ag_half(slicesH[0], 0)
                    elif produce_out:
                        for h in (0, 1):
                            w = 2 * g + h
                            wb = slice(w * P, (w + 1) * P)
                            pst = psT.tile([P, P], f16, tag="pst")
                            nc.tensor.transpose(out=pst[:, :CLS],
                                                in_=acc[:CLS, wb],
                                                identity=ident_t[:CLS, :CLS])
                            nm = stp.tile([P, 1], fp, tag="nm")
                            nc.vector.tensor_reduce(
                                out=nm[:], in_=pst[:, :CLS], op=Alu.max,
                                axis=mybir.AxisListType.X, negate=True)
                            ex = stp.tile([P, CLS], fp, tag="ex")
                            ssum = stp.tile([P, 1], fp, tag="ssum")
                            nc.scalar.activation(out=ex[:], in_=pst[:, :CLS],
                                                 func=Act.Exp, bias=nm[:, 0:1],
                                                 accum_out=ssum[:, 0:1])
                            lse = stp.tile([P, 1], fp, tag="lse")
                            nc.scalar.activation(out=lse[:], in_=ssum[:],
                                                 func=Act.Ln)
                            res = stp.tile([P, CLS], fp, tag="res")
                            nc.vector.tensor_scalar(
                                out=res[:], in0=pst[:, :CLS],
                                scalar1=nm[:, 0:1], scalar2=lse[:, 0:1],
                                op0=Alu.add, op1=Alu.subtract)
                            nc.scalar.dma_start(
                                out=out_d[w * P:(w + 1) * P, :], in_=res[:])
                    if want_slice and g == WSPLIT // 2 - 1:
                        tabs[0] = ag_half(slices[0], 0)
                if want_slice:
                    tabs[1] = ag_half(slices[1], 1)
                return tx_new, tabs

            for l, (Wt, b_t, fo) in enumerate(
                    ((w0_t, b0_t, HID), (w1_t, b1_t, HID), (w2_t, b2_t, CLS))):
                if l * 10 >= stage:
                    break
                last = l == 2
                acc = accp.tile([P, SL], f16, tag="acc")
                # ---- k=0 term: acc = W[0].T @ tx0 + b ----
                for g in range(NPAIR):
                    ch = slice(2 * g * P, (2 * g + 2) * P)
                    pw = psW.tile([P, 2 * P], fp, tag="pw")
                    nc.tensor.matmul(out=pw[:fo, :], lhsT=Wt[:, 0, :fo],
                                     rhs=tx0[:, ch], start=True, stop=True)
                    nc.vector.tensor_scalar(
                        out=acc[:fo, ch], in0=pw[:fo, :],
                        scalar1=b_t[:fo, 0:1], scalar2=None, op0=Alu.add)
                # ---- k=1..3 ----
                if stage < l * 10 + 2:
                    break
                tx1, t1 = spmm(tabA_in, tabB_in, None, Wt, b_t, fo, acc,
                               1, True)
                if stage < l * 10 + 4:
                    break
                tx2, t2 = spmm(t1[0][:, :], t1[1][:, :], tx0,
                               Wt, b_t, fo, acc, 2, True)
                if stage < l * 10 + 6:
                    break
                if not last:
                    hT = txp.tile([P, SL], f16, tag="tx")
                    slicesH = [dramp.tile([HROWSA if hh == 0 else HROWSB, F],
                                          f16, tag="sl", name=f"slh{hh}")
                               for hh in (0, 1)]
                    tabsH = [None, None]
                    spmm(t2[0][:, :], t2[1][:, :], tx1,
                         Wt, b_t, fo, acc, 3, False,
                         produce_h=(hT, slicesH, tabsH))
                    tabsH[1] = ag_half(slicesH[1], 1)
                    tx0 = hT
                    tabA_in, tabB_in = tabsH[0][:, :], tabsH[1][:, :]
                else:
                    spmm(t2[0][:, :], t2[1][:, :], tx1,
                         Wt, b_t, fo, acc, 3, False, produce_out=True)

    nc.compile()
    return nc


_CACHE = {}


def _get_nc(CA, CB, stage=99):
    key = (CA, CB, stage)
    if key not in _CACHE:
        _CACHE[key] = _build(CA, CB, stage)
    return _CACHE[key]


def _run(x, edge_src, edge_dst, W0, b0, W1, b1, W2, b2,
         trace=False, trace_cores=None, stage=99):
    from concourse import bass_utils

    n = x.shape[0]
    pre = _preprocess(edge_src, edge_dst, n)
    perm, CA, CB = pre["perm"], pre["CA"], pre["CB"]
    dinv = pre["dinv_new"]  # by new node id

    x = np.asarray(x, np.float32)
    x_pad = np.zeros((NPAD, F), np.float32)
    x_pad[perm] = x

    # prescaled table in half-major row layout
    rows = _table_row(np.arange(NPAD))
    xfull = np.zeros((NPAD, F), np.float16)
    xfull[rows] = (x_pad * dinv[:, None]).astype(np.float16)

    w0t = np.transpose(np.asarray(W0, np.float32), (1, 0, 2)).astype(np.float16)
    w1t = np.transpose(np.asarray(W1, np.float32), (1, 0, 2)).astype(np.float16)
    w2t = np.transpose(np.asarray(W2, np.float32), (1, 0, 2)).astype(np.float16)
    w0t, w1t, w2t = (np.ascontiguousarray(a) for a in (w0t, w1t, w2t))
    iota = np.ascontiguousarray(
        np.broadcast_to(np.arange(P, dtype=np.float16), (P, P)))
    ident = np.eye(P, dtype=np.float16)

    in_maps = []
    for c in range(CORES):
        rows_c = slice(c * SL, (c + 1) * SL)
        dinv_c = dinv[rows_c]  # [6400] by local node id (w*128+p)
        in_maps.append(dict(
            xT=np.ascontiguousarray(x_pad[rows_c].T).astype(np.float16),
            xfull=xfull,
            ixA=pre["idxA"][c], ixB=pre["idxB"][c],
            dl=pre["dl"][c].astype(np.float16),
            d1=np.ascontiguousarray(
                np.broadcast_to(-dinv_c, (P, SL))).astype(np.float16),
            d2=np.ascontiguousarray(
                np.broadcast_to(-2.0 * dinv_c, (P, SL))).astype(np.float16),
            dinvP=np.ascontiguousarray(
                dinv_c.reshape(NW, P).T).astype(np.float32),
            w0t=w0t, w1t=w1t, w2t=w2t,
            b0=np.asarray(b0, np.float32).reshape(HID, 1),
            b1=np.asarray(b1, np.float32).reshape(HID, 1),
            b2=np.asarray(b2, np.float32).reshape(CLS, 1),
            iota=iota, ident=ident,
        ))

    nc = _get_nc(CA, CB, stage)
    kw = {}
    if trace:
        kw = dict(trace=True,
                  trace_cores=trace_cores if trace_cores is not None else [0])
    res = bass_utils.run_bass_kernel_spmd(nc, in_maps,
                                          core_ids=list(range(CORES)), **kw)

    full = np.concatenate([res.results[c]["out"] for c in range(CORES)],
                          axis=0)
    out = full[perm]
    return out.astype(np.float32), res


def kernel(x, edge_src, edge_dst, W0, b0, W1, b1, W2, b2):
    out, _ = _run(x, edge_src, edge_dst, W0, b0, W1, b1, W2, b2)
    return out


# revision 10
# speedup vs baseline: 1.5641x; 1.5641x over previous
"""ChebConv GNN (3 layers, K=4) on 8 Trainium2 NeuronCores.

Pull-mode graph-parallel SpMM: nodes are partitioned across the 8 cores
(LPT on in-degree into 400 windows of 128 dst nodes). Each SpMM gathers
source rows from a replicated fp16 node-major table in HBM (SWDGE
dma_gather, persistent SBUF-resident indices, 4 queues), segment-sums
them per 128-dst window with a BINARY one-hot matmul on the TensorEngine
(the ChebConv edge weights -dinv[s]*dinv[d] are factorized: dinv[s] is
pre-folded into the table values, dinv[d] applied as a per-window
post-scale), and per-core slices are re-replicated with half-table
AllGather chunks, the first fired mid-SpMM to overlap the collective
with compute.

Compute layout is feature-major ([feature, node]); node-major table
slices are produced with PE transposes, the dinv prescale fused into the
PSUM-drain copy. All tables/messages/weights are fp16 with fp32 PSUM
accumulation.
"""

import numpy as np

# ---------------- problem constants (hardcoded per contract) ----------------
N, E = 50000, 800000
F, HID, CLS, K = 128, 128, 40, 4
P = 128
CORES = 8
NW = 50                 # dst windows per core
SL = NW * P             # 6400 nodes per core
NPAD = CORES * SL       # 51200 padded node count
NPAIR = NW // 2         # window pairs per core
WSPLIT = 30             # windows 0..29 -> half A (60%), rest -> half B
HALFA = CORES * WSPLIT * P      # 30720 rows (< 32768, int16-safe)
HALFB = NPAD - HALFA            # 20480 rows
HROWSA = WSPLIT * P             # 3840 slice rows (half A, per core)
HROWSB = (NW - WSPLIT) * P      # 2560 slice rows (half B, per core)
GDELAY = 7              # pairs of A-gather lookahead past B-gathers


# ---------------- host preprocessing ----------------
def _lpt_windows(indeg, n_windows, cap):
    """Assign nodes to windows (cap nodes each), balancing in-degree sums.
    Returns perm: old node id -> new node id."""
    import heapq
    order = np.argsort(-indeg, kind="stable")
    heap = [(0, wi) for wi in range(n_windows)]
    heapq.heapify(heap)
    counts = np.zeros(n_windows, np.int64)
    perm = np.empty(len(indeg), np.int64)
    for old in order:
        while True:
            load, wi = heapq.heappop(heap)
            if counts[wi] < cap:
                break
        perm[old] = wi * cap + counts[wi]
        counts[wi] += 1
        if counts[wi] < cap:
            heapq.heappush(heap, (load + int(indeg[old]), wi))
    return perm


def _table_row(nid):
    """New node id -> row in the half-major table layout:
    [half h][core c][local window][pos 128]."""
    c = nid // SL
    w = (nid % SL) // P
    p = nid % P
    h = (w >= WSPLIT).astype(np.int64) if hasattr(w, 'astype') else int(w >= WSPLIT)
    wl = w - h * WSPLIT
    hrows = np.where(h, HROWSB, HROWSA) if hasattr(w, 'astype') else (HROWSB if h else HROWSA)
    return h * HALFA + c * hrows + wl * P + p


def _preprocess(edge_src, edge_dst, n):
    es = np.asarray(edge_src, np.int64)
    ed = np.asarray(edge_dst, np.int64)
    deg = np.bincount(es, minlength=n).astype(np.float32)
    dinv = np.where(deg > 0, 1.0 / np.sqrt(np.maximum(deg, 1.0)), 0.0).astype(
        np.float32
    )

    indeg = np.bincount(ed, minlength=n)
    perm = _lpt_windows(indeg, CORES * NW, P)  # old -> new

    dinv_new = np.zeros(NPAD, np.float32)
    dinv_new[perm] = dinv

    nsrc = perm[es]
    ndst = perm[ed]
    core_e = ndst // SL
    win_e = (ndst % SL) // P
    dloc_e = (ndst % P).astype(np.int64)
    srow = _table_row(nsrc)
    half_e = (srow >= HALFA).astype(np.int64)
    lrow_e = srow - half_e * HALFA  # local row within half table (int16-safe)

    # group edges by (core, win, half)
    gkey = (core_e * NW + win_e) * 2 + half_e
    ngroups = CORES * NW * 2
    order = np.argsort(gkey, kind="stable")
    gkey_s = gkey[order]
    counts = np.bincount(gkey_s, minlength=ngroups)
    starts = np.concatenate([[0], np.cumsum(counts)[:-1]])
    rank = np.arange(len(es)) - starts[gkey_s]

    cnts = counts.reshape(CORES, NW, 2)
    CA = max(int(np.ceil(cnts[:, :, 0].max() / P)), 1)
    CB = max(int(np.ceil(cnts[:, :, 1].max() / P)), 1)
    CW = CA + CB

    capa = {0: CA * P, 1: CB * P}
    idx_pad = {h: np.zeros((CORES, NW, capa[h]), np.int16) for h in (0, 1)}
    dl_pad = np.full((CORES, NW, CW, P), 255.0, np.float32)

    ce, we, he = core_e[order], win_e[order], half_e[order]
    de, ie = dloc_e[order], lrow_e[order]
    for h in (0, 1):
        m = he == h
        idx_pad[h][ce[m], we[m], rank[m]] = ie[m].astype(np.int16)
        coff = rank[m] // P + (0 if h == 0 else CA)
        dl_pad[ce[m], we[m], coff, rank[m] % P] = de[m]

    # persistent wrapped idx arrays: per pair g, A block = [win 2g cols,
    # win 2g+1 cols] (2*CA*128 idxs), concatenated over the 25 pairs.
    def wrap(idxs):  # [cores, npair, L] -> [cores, 128, npair*(L//16)]
        c, g, L = idxs.shape
        a = idxs.reshape(c, g, L // 16, 16).transpose(0, 1, 3, 2)
        a = np.tile(a, (1, 1, 8, 1))  # [c, g, 128, L//16]
        return np.concatenate([a[:, i] for i in range(g)], axis=2).copy()

    idxA = wrap(idx_pad[0].reshape(CORES, NPAIR, 2 * CA * P))
    idxB = wrap(idx_pad[1].reshape(CORES, NPAIR, 2 * CB * P))

    dl_arr = dl_pad.transpose(0, 3, 1, 2).reshape(CORES, P, NW * CW).copy()

    return dict(perm=perm, dinv_new=dinv_new, CA=CA, CB=CB, CW=CW,
                idxA=idxA, idxB=idxB, dl=dl_arr)


# ---------------- device kernel ----------------
def _build(CA, CB, stage=99):
    import concourse.bass as bass
    import concourse.bacc as bacc
    import concourse.tile as tile
    import concourse.mybir as mybir
    import dataclasses

    CW = CA + CB
    fp = mybir.dt.float32
    f16 = mybir.dt.float16
    Alu = mybir.AluOpType
    Act = mybir.ActivationFunctionType

    nc = bacc.Bacc("TRN2", target_bir_lowering=False, debug=False,
                   num_devices=CORES, num_swdge_queues=4)

    # -------- I/O --------
    xT_d = nc.dram_tensor("xT", [P, SL], f16, kind="ExternalInput")
    xfull_d = nc.dram_tensor("xfull", [NPAD, F], f16, kind="ExternalInput")
    ixA_d = nc.dram_tensor("ixA", [P, NPAIR * 2 * CA * 8], mybir.dt.int16,
                           kind="ExternalInput")
    ixB_d = nc.dram_tensor("ixB", [P, NPAIR * 2 * CB * 8], mybir.dt.int16,
                           kind="ExternalInput")
    dl_d = nc.dram_tensor("dl", [P, NW * CW], f16, kind="ExternalInput")
    d1_d = nc.dram_tensor("d1", [P, SL], f16, kind="ExternalInput")
    d2_d = nc.dram_tensor("d2", [P, SL], f16, kind="ExternalInput")
    dinvP_d = nc.dram_tensor("dinvP", [P, NW], fp, kind="ExternalInput")
    w0_d = nc.dram_tensor("w0t", [P, K, HID], f16, kind="ExternalInput")
    w1_d = nc.dram_tensor("w1t", [P, K, HID], f16, kind="ExternalInput")
    w2_d = nc.dram_tensor("w2t", [P, K, CLS], f16, kind="ExternalInput")
    b0_d = nc.dram_tensor("b0", [HID, 1], fp, kind="ExternalInput")
    b1_d = nc.dram_tensor("b1", [HID, 1], fp, kind="ExternalInput")
    b2_d = nc.dram_tensor("b2", [CLS, 1], fp, kind="ExternalInput")
    iota_d = nc.dram_tensor("iota", [P, P], f16, kind="ExternalInput")
    ident_d = nc.dram_tensor("ident", [P, P], f16, kind="ExternalInput")
    out_d = nc.dram_tensor("out", [SL, CLS], fp, kind="ExternalOutput")

    def bmid(ap, n):  # [128, X] -> [128, n, X], middle stride 0
        return dataclasses.replace(ap, ap=[ap.ap[0], [0, n], ap.ap[1]])

    def blast(ap, n):  # [128, X] -> [128, X, n], last stride 0
        return dataclasses.replace(ap, ap=[ap.ap[0], ap.ap[1], [0, n]])

    nA, nB = 2 * CA * P, 2 * CB * P

    with tile.TileContext(nc) as tc:
        with (
            tc.tile_pool(name="const", bufs=1) as constp,
            tc.tile_pool(name="tx", bufs=3) as txp,
            tc.tile_pool(name="acc", bufs=1) as accp,
            tc.tile_pool(name="g", bufs=9) as gp,
            tc.tile_pool(name="m", bufs=2) as mp,
            tc.tile_pool(name="tmp", bufs=3) as tmpp,
            tc.tile_pool(name="st", bufs=6) as stp,
            tc.tile_pool(name="psA", bufs=3, space="PSUM") as psA,
            tc.tile_pool(name="psT", bufs=3, space="PSUM") as psT,
            tc.tile_pool(name="psW", bufs=2, space="PSUM") as psW,
            tc.tile_pool(name="dram", bufs=4, space="DRAM") as dramp,
            tc.tile_pool(name="tabs", bufs=4, space="DRAM") as tabp,
        ):
            # -------- constants --------
            ixA_t = constp.tile([P, NPAIR * 2 * CA * 8], mybir.dt.int16)
            ixB_t = constp.tile([P, NPAIR * 2 * CB * 8], mybir.dt.int16)
            dl_t = constp.tile([P, NW * CW], f16)
            d1_t = constp.tile([P, SL], f16)
            d2_t = constp.tile([P, SL], f16)
            dinvP_t = constp.tile([P, NW], fp)
            iota_t = constp.tile([P, P], f16)
            ident_t = constp.tile([P, P], f16)
            w0_t = constp.tile([P, K, HID], f16)
            w1_t = constp.tile([P, K, HID], f16)
            w2_t = constp.tile([P, K, CLS], f16)
            b0_t = constp.tile([HID, 1], fp)
            b1_t = constp.tile([HID, 1], fp)
            b2_t = constp.tile([CLS, 1], fp)
            for t, d in ((ixA_t, ixA_d), (ixB_t, ixB_d), (dl_t, dl_d),
                         (d1_t, d1_d), (d2_t, d2_d), (dinvP_t, dinvP_d),
                         (iota_t, iota_d), (ident_t, ident_d),
                         (w0_t, w0_d), (w1_t, w1_d), (w2_t, w2_d),
                         (b0_t, b0_d), (b1_t, b1_d), (b2_t, b2_d)):
                nc.sync.dma_start(out=t[:], in_=d[:])

            tx0 = txp.tile([P, SL], f16, tag="tx")
            nc.sync.dma_start(out=tx0[:], in_=xT_d[:, :])

            tabA_in = xfull_d[0:HALFA, :]
            tabB_in = xfull_d[HALFA:NPAD, :]

            def ag_half(slice_h, h):
                """AllGather one half-slice into a fresh half-table."""
                tab = tabp.tile([HALFA if h == 0 else HALFB, F], f16,
                                tag="tab", addr_space="Shared",
                                name=f"tab{h}")
                nc.gpsimd.collective_compute(
                    "AllGather", Alu.bypass,
                    replica_groups=[list(range(CORES))],
                    ins=[slice_h[:, :].opt()],
                    outs=[tab[:, :].opt()])
                return tab

            def store_win(src_fm, w, slices):
                """Transpose window w of feature-major src, scale by dinv,
                store node-major into the half slice tile."""
                wb = slice(w * P, (w + 1) * P)
                pst = psT.tile([P, P], f16, tag="pst")
                nc.tensor.transpose(out=pst[:], in_=src_fm[:, wb],
                                    identity=ident_t[:])
                st = stp.tile([P, P], f16, tag="st")
                nc.scalar.activation(out=st[:], in_=pst[:], func=Act.Copy,
                                     scale=dinvP_t[:, w:w + 1])
                h = int(w >= WSPLIT)
                wl = w - h * WSPLIT
                nc.sync.dma_start(out=slices[h][wl * P:(wl + 1) * P, :],
                                  in_=st[:])

            def spmm(tabA, tabB, tx_prev2, Wt, b_t, fo, acc, k,
                     want_slice, produce_h=None, produce_out=False):
                """One lhat hop (k-th Chebyshev term).
                want_slice: make the dinv-scaled table of tx_new + AG.
                produce_h: (hT, slicesH, tabH) -> relu(acc) table (k=3 hop
                of non-last layers).  produce_out: last-layer epilogue."""
                tx_new = txp.tile([P, SL], f16, tag="tx")
                slices = None
                tabs = [None, None]
                if want_slice:
                    slices = [dramp.tile([HROWSA if hh == 0 else HROWSB, F],
                                         f16, tag="sl", name=f"sl{hh}")
                              for hh in (0, 1)]
                Gs = {}

                def emit_A(g):
                    G = gp.tile([P, 2 * CW, P], f16, tag="G", name="G")
                    Gs[g] = G
                    nc.gpsimd.dma_gather(
                        out_ap=G[:, 0:2 * CA, :], in_ap=tabA,
                        idxs_ap=ixA_t[:, g * 2 * CA * 8:(g + 1) * 2 * CA * 8],
                        num_idxs=nA, num_idxs_reg=nA, elem_size=P,
                        single_packet=False, queue_num=(2 * g) % 4)

                for g in range(GDELAY):
                    emit_A(g)
                for g in range(NPAIR):
                    G = Gs.pop(g)
                    nc.gpsimd.dma_gather(
                        out_ap=G[:, 2 * CA:2 * CW, :], in_ap=tabB,
                        idxs_ap=ixB_t[:, g * 2 * CB * 8:(g + 1) * 2 * CB * 8],
                        num_idxs=nB, num_idxs_reg=nB, elem_size=P,
                        single_packet=False, queue_num=(2 * g + 1) % 4)
                    if g + GDELAY < NPAIR:
                        emit_A(g + GDELAY)
                    M2 = mp.tile([P, 2 * CW, P], f16, tag="M")
                    nc.vector.tensor_tensor(
                        out=M2[:], in0=bmid(iota_t[:], 2 * CW),
                        in1=blast(dl_t[:, 2 * g * CW:(2 * g + 2) * CW], P),
                        op=Alu.is_equal)
                    for h in (0, 1):
                        w = 2 * g + h
                        wb = slice(w * P, (w + 1) * P)
                        M = M2[:, h * CW:(h + 1) * CW, :]
                        ps = psA.tile([P, P], fp, tag="ps")
                        for c in range(CW):
                            Gsl = (G[:, h * CA + c, :] if c < CA
                                   else G[:, 2 * CA + h * CB + (c - CA), :])
                            nc.tensor.matmul(out=ps[:], lhsT=Gsl,
                                             rhs=M[:, c, :],
                                             start=(c == 0), stop=(c == CW - 1))
                        if tx_prev2 is None:
                            nc.vector.tensor_tensor(
                                out=tx_new[:, wb], in0=ps[:],
                                in1=d1_t[:, wb], op=Alu.mult)
                        else:
                            tm = tmpp.tile([P, P], fp, tag="tm")
                            nc.vector.tensor_tensor(
                                out=tm[:], in0=ps[:], in1=d2_t[:, wb],
                                op=Alu.mult)
                            nc.vector.tensor_tensor(
                                out=tx_new[:, wb], in0=tm[:],
                                in1=tx_prev2[:, wb], op=Alu.subtract)
                        if want_slice:
                            store_win(tx_new, w, slices)
                    # ---- per-pair: W-matmul chunk, acc update, finales ----
                    ch = slice(2 * g * P, (2 * g + 2) * P)
                    pw = psW.tile([P, 2 * P], fp, tag="pw")
                    nc.tensor.matmul(out=pw[:fo, :], lhsT=Wt[:, k, :fo],
                                     rhs=tx_new[:, ch], start=True, stop=True)
                    nc.vector.tensor_tensor(out=acc[:fo, ch],
                                            in0=acc[:fo, ch],
                                            in1=pw[:fo, :], op=Alu.add)
                    if produce_h is not None:
                        hT, slicesH, tabsH = produce_h
                        nc.scalar.activation(out=hT[:, ch], in_=acc[:, ch],
                                             func=Act.Relu)
                        for h in (0, 1):
                            store_win(hT, 2 * g + h, slicesH)
                        if g == WSPLIT // 2 - 1:
                            tabsH[0] = ag_half(slicesH[0], 0)
                    elif produce_out:
                        for h in (0, 1):
                            w = 2 * g + h
                            wb = slice(w * P, (w + 1) * P)
                            pst = psT.tile([P, P], f16, tag="pst")
                            nc.tensor.transpose(out=pst[:, :CLS],
                                                in_=acc[:CLS, wb],
                                                identity=ident_t[:CLS, :CLS])
                            nm = stp.tile([P, 1], fp, tag="nm")
                            nc.vector.tensor_reduce(
                                out=nm[:], in_=pst[:, :CLS], op=Alu.max,
                                axis=mybir.AxisListType.X, negate=True)
                            ex = stp.tile([P, CLS], fp, tag="ex")
                            ssum = stp.tile([P, 1], fp, tag="ssum")
                            nc.scalar.activation(out=ex[:], in_=pst[:, :CLS],
                                                 func=Act.Exp, bias=nm[:, 0:1],
                                                 accum_out=ssum[:, 0:1])
                            lse = stp.tile([P, 1], fp, tag="lse")
                            nc.scalar.activation(out=lse[:], in_=ssum[:],
                                                 func=Act.Ln)
                            res = stp.tile([P, CLS], fp, tag="res")
                            nc.vector.tensor_scalar(
                                out=res[:], in0=pst[:, :CLS],
                                scalar1=nm[:, 0:1], scalar2=lse[:, 0:1],
                                op0=Alu.add, op1=Alu.subtract)
                            nc.scalar.dma_start(
                                out=out_d[w * P:(w + 1) * P, :], in_=res[:])
                    if want_slice and g == WSPLIT // 2 - 1:
                        tabs[0] = ag_half(slices[0], 0)
                if want_slice:
                    tabs[1] = ag_half(slices[1], 1)
                return tx_new, tabs

            for l, (Wt, b_t, fo) in enumerate(
                    ((w0_t, b0_t, HID), (w1_t, b1_t, HID), (w2_t, b2_t, CLS))):
                if l * 10 >= stage:
                    break
                last = l == 2
                acc = accp.tile([P, SL], f16, tag="acc")
                # ---- k=0 term: acc = W[0].T @ tx0 + b ----
                for g in range(NPAIR):
                    ch = slice(2 * g * P, (2 * g + 2) * P)
                    pw = psW.tile([P, 2 * P], fp, tag="pw")
                    nc.tensor.matmul(out=pw[:fo, :], lhsT=Wt[:, 0, :fo],
                                     rhs=tx0[:, ch], start=True, stop=True)
                    nc.vector.tensor_scalar(
                        out=acc[:fo, ch], in0=pw[:fo, :],
                        scalar1=b_t[:fo, 0:1], scalar2=None, op0=Alu.add)
                # ---- k=1..3 ----
                if stage < l * 10 + 2:
                    break
                tx1, t1 = spmm(tabA_in, tabB_in, None, Wt, b_t, fo, acc,
                               1, True)
                if stage < l * 10 + 4:
                    break
                tx2, t2 = spmm(t1[0][:, :], t1[1][:, :], tx0,
                               Wt, b_t, fo, acc, 2, True)
                if stage < l * 10 + 6:
                    break
                if not last:
                    hT = txp.tile([P, SL], f16, tag="tx")
                    slicesH = [dramp.tile([HROWSA if hh == 0 else HROWSB, F],
                                          f16, tag="sl", name=f"slh{hh}")
                               for hh in (0, 1)]
                    tabsH = [None, None]
                    spmm(t2[0][:, :], t2[1][:, :], tx1,
                         Wt, b_t, fo, acc, 3, False,
                         produce_h=(hT, slicesH, tabsH))
                    tabsH[1] = ag_half(slicesH[1], 1)
                    tx0 = hT
                    tabA_in, tabB_in = tabsH[0][:, :], tabsH[1][:, :]
                else:
                    spmm(t2[0][:, :], t2[1][:, :], tx1,
                         Wt, b_t, fo, acc, 3, False, produce_out=True)

    nc.compile()
    return nc


_CACHE = {}


def _get_nc(CA, CB, stage=99):
    key = (CA, CB, stage)
    if key not in _CACHE:
        _CACHE[key] = _build(CA, CB, stage)
    return _CACHE[key]


def _run(x, edge_src, edge_dst, W0, b0, W1, b1, W2, b2,
         trace=False, trace_cores=None, stage=99):
    from concourse import bass_utils

    n = x.shape[0]
    pre = _preprocess(edge_src, edge_dst, n)
    perm, CA, CB = pre["perm"], pre["CA"], pre["CB"]
    dinv = pre["dinv_new"]  # by new node id

    x = np.asarray(x, np.float32)
    x_pad = np.zeros((NPAD, F), np.float32)
    x_pad[perm] = x

    # prescaled table in half-major row layout
    rows = _table_row(np.arange(NPAD))
    xfull = np.zeros((NPAD, F), np.float16)
    xfull[rows] = (x_pad * dinv[:, None]).astype(np.float16)

    w0t = np.transpose(np.asarray(W0, np.float32), (1, 0, 2)).astype(np.float16)
    w1t = np.transpose(np.asarray(W1, np.float32), (1, 0, 2)).astype(np.float16)
    w2t = np.transpose(np.asarray(W2, np.float32), (1, 0, 2)).astype(np.float16)
    w0t, w1t, w2t = (np.ascontiguousarray(a) for a in (w0t, w1t, w2t))
    iota = np.ascontiguousarray(
        np.broadcast_to(np.arange(P, dtype=np.float16), (P, P)))
    ident = np.eye(P, dtype=np.float16)

    in_maps = []
    for c in range(CORES):
        rows_c = slice(c * SL, (c + 1) * SL)
        dinv_c = dinv[rows_c]  # [6400] by local node id (w*128+p)
        in_maps.append(dict(
            xT=np.ascontiguousarray(x_pad[rows_c].T).astype(np.float16),
            xfull=xfull,
            ixA=pre["idxA"][c], ixB=pre["idxB"][c],
            dl=pre["dl"][c].astype(np.float16),
            d1=np.ascontiguousarray(
                np.broadcast_to(-dinv_c, (P, SL))).astype(np.float16),
            d2=np.ascontiguousarray(
                np.broadcast_to(-2.0 * dinv_c, (P, SL))).astype(np.float16),
            dinvP=np.ascontiguousarray(
                dinv_c.reshape(NW, P).T).astype(np.float32),
            w0t=w0t, w1t=w1t, w2t=w2t,
            b0=np.asarray(b0, np.float32).reshape(HID, 1),
            b1=np.asarray(b1, np.float32).reshape(HID, 1),
            b2=np.asarray(b2, np.float32).reshape(CLS, 1),
            iota=iota, ident=ident,
        ))

    nc = _get_nc(CA, CB, stage)
    kw = {}
    if trace:
        kw = dict(trace=True,
                  trace_cores=trace_cores if trace_cores is not None else [0])
    res = bass_utils.run_bass_kernel_spmd(nc, in_maps,
                                          core_ids=list(range(CORES)), **kw)

    full = np.concatenate([res.results[c]["out"] for c in range(CORES)],
                          axis=0)
    out = full[perm]
    return out.astype(np.float32), res


def kernel(x, edge_src, edge_dst, W0, b0, W1, b1, W2, b2):
    out, _ = _run(x, edge_src, edge_dst, W0, b0, W1, b1, W2, b2)
    return out


# revision 11
# speedup vs baseline: 1.6356x; 1.0457x over previous
"""ChebConv GNN (3 layers, K=4) on 8 Trainium2 NeuronCores.

Pull-mode graph-parallel SpMM: nodes are partitioned across the 8 cores
(LPT on in-degree into 400 windows of 128 dst nodes). Each SpMM gathers
source rows from a replicated fp16 node-major table in HBM (SWDGE
dma_gather, persistent SBUF-resident indices, 4 queues), segment-sums
them per 128-dst window with a BINARY one-hot matmul on the TensorEngine
(the ChebConv edge weights -dinv[s]*dinv[d] are factorized: dinv[s] is
pre-folded into the table values, dinv[d] applied as a per-window
post-scale), and per-core slices are re-replicated with half-table
AllGather chunks, the first fired mid-SpMM to overlap the collective
with compute.

Compute layout is feature-major ([feature, node]); node-major table
slices are produced with PE transposes, the dinv prescale fused into the
PSUM-drain copy. All tables/messages/weights are fp16 with fp32 PSUM
accumulation.
"""

import numpy as np

# ---------------- problem constants (hardcoded per contract) ----------------
N, E = 50000, 800000
F, HID, CLS, K = 128, 128, 40, 4
P = 128
CORES = 8
NW = 50                 # dst windows per core
SL = NW * P             # 6400 nodes per core
NPAD = CORES * SL       # 51200 padded node count
NPAIR = NW // 2         # window pairs per core
WSPLIT = 30             # windows 0..29 -> half A (60%), rest -> half B
HALFA = CORES * WSPLIT * P      # 30720 rows (< 32768, int16-safe)
HALFB = NPAD - HALFA            # 20480 rows
HROWSA = WSPLIT * P             # 3840 slice rows (half A, per core)
HROWSB = (NW - WSPLIT) * P      # 2560 slice rows (half B, per core)
GDELAY = 5              # pairs of A-gather lookahead past B-gathers


# ---------------- host preprocessing ----------------
def _lpt_windows(indeg, n_windows, cap):
    """Assign nodes to windows (cap nodes each), balancing in-degree sums.
    Returns perm: old node id -> new node id."""
    import heapq
    order = np.argsort(-indeg, kind="stable")
    heap = [(0, wi) for wi in range(n_windows)]
    heapq.heapify(heap)
    counts = np.zeros(n_windows, np.int64)
    perm = np.empty(len(indeg), np.int64)
    for old in order:
        while True:
            load, wi = heapq.heappop(heap)
            if counts[wi] < cap:
                break
        perm[old] = wi * cap + counts[wi]
        counts[wi] += 1
        if counts[wi] < cap:
            heapq.heappush(heap, (load + int(indeg[old]), wi))
    return perm


def _table_row(nid):
    """New node id -> row in the half-major table layout:
    [half h][core c][local window][pos 128]."""
    c = nid // SL
    w = (nid % SL) // P
    p = nid % P
    h = (w >= WSPLIT).astype(np.int64) if hasattr(w, 'astype') else int(w >= WSPLIT)
    wl = w - h * WSPLIT
    hrows = np.where(h, HROWSB, HROWSA) if hasattr(w, 'astype') else (HROWSB if h else HROWSA)
    return h * HALFA + c * hrows + wl * P + p


def _preprocess(edge_src, edge_dst, n):
    es = np.asarray(edge_src, np.int64)
    ed = np.asarray(edge_dst, np.int64)
    deg = np.bincount(es, minlength=n).astype(np.float32)
    dinv = np.where(deg > 0, 1.0 / np.sqrt(np.maximum(deg, 1.0)), 0.0).astype(
        np.float32
    )

    indeg = np.bincount(ed, minlength=n)
    perm = _lpt_windows(indeg, CORES * NW, P)  # old -> new

    dinv_new = np.zeros(NPAD, np.float32)
    dinv_new[perm] = dinv

    nsrc = perm[es]
    ndst = perm[ed]
    core_e = ndst // SL
    win_e = (ndst % SL) // P
    dloc_e = (ndst % P).astype(np.int64)
    srow = _table_row(nsrc)
    half_e = (srow >= HALFA).astype(np.int64)
    lrow_e = srow - half_e * HALFA  # local row within half table (int16-safe)

    # group edges by (core, win, half)
    gkey = (core_e * NW + win_e) * 2 + half_e
    ngroups = CORES * NW * 2
    order = np.argsort(gkey, kind="stable")
    gkey_s = gkey[order]
    counts = np.bincount(gkey_s, minlength=ngroups)
    starts = np.concatenate([[0], np.cumsum(counts)[:-1]])
    rank = np.arange(len(es)) - starts[gkey_s]

    cnts = counts.reshape(CORES, NW, 2)
    CA = max(int(np.ceil(cnts[:, :, 0].max() / P)), 1)
    CB = max(int(np.ceil(cnts[:, :, 1].max() / P)), 1)
    CW = CA + CB

    capa = {0: CA * P, 1: CB * P}
    idx_pad = {h: np.zeros((CORES, NW, capa[h]), np.int16) for h in (0, 1)}
    dl_pad = np.full((CORES, NW, CW, P), 255.0, np.float32)

    ce, we, he = core_e[order], win_e[order], half_e[order]
    de, ie = dloc_e[order], lrow_e[order]
    for h in (0, 1):
        m = he == h
        idx_pad[h][ce[m], we[m], rank[m]] = ie[m].astype(np.int16)
        coff = rank[m] // P + (0 if h == 0 else CA)
        dl_pad[ce[m], we[m], coff, rank[m] % P] = de[m]

    # persistent wrapped idx arrays: per pair g, A block = [win 2g cols,
    # win 2g+1 cols] (2*CA*128 idxs), concatenated over the 25 pairs.
    def wrap(idxs):  # [cores, npair, L] -> [cores, 128, npair*(L//16)]
        c, g, L = idxs.shape
        a = idxs.reshape(c, g, L // 16, 16).transpose(0, 1, 3, 2)
        a = np.tile(a, (1, 1, 8, 1))  # [c, g, 128, L//16]
        return np.concatenate([a[:, i] for i in range(g)], axis=2).copy()

    idxA = wrap(idx_pad[0].reshape(CORES, NPAIR, 2 * CA * P))
    idxB = wrap(idx_pad[1].reshape(CORES, NPAIR, 2 * CB * P))

    dl_arr = dl_pad.transpose(0, 3, 1, 2).reshape(CORES, P, NW * CW).copy()

    return dict(perm=perm, dinv_new=dinv_new, CA=CA, CB=CB, CW=CW,
                idxA=idxA, idxB=idxB, dl=dl_arr)


# ---------------- device kernel ----------------
def _build(CA, CB, stage=99):
    import concourse.bass as bass
    import concourse.bacc as bacc
    import concourse.tile as tile
    import concourse.mybir as mybir
    import dataclasses

    CW = CA + CB
    fp = mybir.dt.float32
    f16 = mybir.dt.float16
    Alu = mybir.AluOpType
    Act = mybir.ActivationFunctionType

    nc = bacc.Bacc("TRN2", target_bir_lowering=False, debug=False,
                   num_devices=CORES, num_swdge_queues=4)

    # -------- I/O --------
    xT_d = nc.dram_tensor("xT", [P, SL], f16, kind="ExternalInput")
    xfull_d = nc.dram_tensor("xfull", [NPAD, F], f16, kind="ExternalInput")
    ixA_d = nc.dram_tensor("ixA", [P, NPAIR * 2 * CA * 8], mybir.dt.int16,
                           kind="ExternalInput")
    ixB_d = nc.dram_tensor("ixB", [P, NPAIR * 2 * CB * 8], mybir.dt.int16,
                           kind="ExternalInput")
    dl_d = nc.dram_tensor("dl", [P, NW * CW], f16, kind="ExternalInput")
    d1_d = nc.dram_tensor("d1", [P, SL], f16, kind="ExternalInput")
    d2_d = nc.dram_tensor("d2", [P, SL], f16, kind="ExternalInput")
    dinvP_d = nc.dram_tensor("dinvP", [P, NW], fp, kind="ExternalInput")
    w0_d = nc.dram_tensor("w0t", [P, K, HID], f16, kind="ExternalInput")
    w1_d = nc.dram_tensor("w1t", [P, K, HID], f16, kind="ExternalInput")
    w2_d = nc.dram_tensor("w2t", [P, K, CLS], f16, kind="ExternalInput")
    b0_d = nc.dram_tensor("b0", [HID, 1], fp, kind="ExternalInput")
    b1_d = nc.dram_tensor("b1", [HID, 1], fp, kind="ExternalInput")
    b2_d = nc.dram_tensor("b2", [CLS, 1], fp, kind="ExternalInput")
    iota_d = nc.dram_tensor("iota", [P, P], f16, kind="ExternalInput")
    ident_d = nc.dram_tensor("ident", [P, P], f16, kind="ExternalInput")
    out_d = nc.dram_tensor("out", [SL, CLS], fp, kind="ExternalOutput")

    def bmid(ap, n):  # [128, X] -> [128, n, X], middle stride 0
        return dataclasses.replace(ap, ap=[ap.ap[0], [0, n], ap.ap[1]])

    def blast(ap, n):  # [128, X] -> [128, X, n], last stride 0
        return dataclasses.replace(ap, ap=[ap.ap[0], ap.ap[1], [0, n]])

    nA, nB = 2 * CA * P, 2 * CB * P

    with tile.TileContext(nc) as tc:
        with (
            tc.tile_pool(name="const", bufs=1) as constp,
            tc.tile_pool(name="tx", bufs=3) as txp,
            tc.tile_pool(name="acc", bufs=1) as accp,
            tc.tile_pool(name="g", bufs=7) as gp,
            tc.tile_pool(name="m", bufs=2) as mp,
            tc.tile_pool(name="tmp", bufs=3) as tmpp,
            tc.tile_pool(name="st", bufs=6) as stp,
            tc.tile_pool(name="psA", bufs=3, space="PSUM") as psA,
            tc.tile_pool(name="psT", bufs=2, space="PSUM") as psT,
            tc.tile_pool(name="psW", bufs=2, space="PSUM") as psW,
            tc.tile_pool(name="dram", bufs=4, space="DRAM") as dramp,
            tc.tile_pool(name="tabs", bufs=4, space="DRAM") as tabp,
        ):
            # -------- constants --------
            ixA_t = constp.tile([P, NPAIR * 2 * CA * 8], mybir.dt.int16)
            ixB_t = constp.tile([P, NPAIR * 2 * CB * 8], mybir.dt.int16)
            dl_t = constp.tile([P, NW * CW], f16)
            d1_t = constp.tile([P, SL], f16)
            d2_t = constp.tile([P, SL], f16)
            dinvP_t = constp.tile([P, NW], fp)
            iota_t = constp.tile([P, P], f16)
            ident_t = constp.tile([P, P], f16)
            w0_t = constp.tile([P, K, HID], f16)
            w1_t = constp.tile([P, K, HID], f16)
            w2_t = constp.tile([P, K, CLS], f16)
            b0_t = constp.tile([HID, 1], fp)
            b1_t = constp.tile([HID, 1], fp)
            b2_t = constp.tile([CLS, 1], fp)
            for t, d in ((ixA_t, ixA_d), (ixB_t, ixB_d), (dl_t, dl_d),
                         (d1_t, d1_d), (d2_t, d2_d), (dinvP_t, dinvP_d),
                         (iota_t, iota_d), (ident_t, ident_d),
                         (w0_t, w0_d), (w1_t, w1_d), (w2_t, w2_d),
                         (b0_t, b0_d), (b1_t, b1_d), (b2_t, b2_d)):
                nc.sync.dma_start(out=t[:], in_=d[:])

            tx0 = txp.tile([P, SL], f16, tag="tx")
            nc.sync.dma_start(out=tx0[:], in_=xT_d[:, :])

            tabA_in = xfull_d[0:HALFA, :]
            tabB_in = xfull_d[HALFA:NPAD, :]

            def ag_half(slice_h, h):
                """AllGather one half-slice into a fresh half-table."""
                tab = tabp.tile([HALFA if h == 0 else HALFB, F], f16,
                                tag="tab", addr_space="Shared",
                                name=f"tab{h}")
                nc.gpsimd.collective_compute(
                    "AllGather", Alu.bypass,
                    replica_groups=[list(range(CORES))],
                    ins=[slice_h[:, :].opt()],
                    outs=[tab[:, :].opt()])
                return tab

            def store_win(src_fm, w, slices):
                """Transpose window w of feature-major src, scale by dinv,
                store node-major into the half slice tile."""
                wb = slice(w * P, (w + 1) * P)
                pst = psT.tile([P, P], f16, tag="pst")
                nc.tensor.transpose(out=pst[:], in_=src_fm[:, wb],
                                    identity=ident_t[:])
                st = stp.tile([P, P], f16, tag="st")
                nc.scalar.activation(out=st[:], in_=pst[:], func=Act.Copy,
                                     scale=dinvP_t[:, w:w + 1])
                h = int(w >= WSPLIT)
                wl = w - h * WSPLIT
                nc.sync.dma_start(out=slices[h][wl * P:(wl + 1) * P, :],
                                  in_=st[:])

            def spmm(tabA, tabB, tx_prev2, Wt, b_t, fo, acc, k,
                     want_slice, produce_h=None, produce_out=False):
                """One lhat hop (k-th Chebyshev term).
                want_slice: make the dinv-scaled table of tx_new + AG.
                produce_h: (hT, slicesH, tabH) -> relu(acc) table (k=3 hop
                of non-last layers).  produce_out: last-layer epilogue."""
                tx_new = txp.tile([P, SL], f16, tag="tx")
                slices = None
                tabs = [None, None]
                if want_slice:
                    slices = [dramp.tile([HROWSA if hh == 0 else HROWSB, F],
                                         f16, tag="sl", name=f"sl{hh}")
                              for hh in (0, 1)]
                Gs = {}

                def emit_A(g):
                    G = gp.tile([P, 2 * CW, P], f16, tag="G", name="G")
                    Gs[g] = G
                    nc.gpsimd.dma_gather(
                        out_ap=G[:, 0:2 * CA, :], in_ap=tabA,
                        idxs_ap=ixA_t[:, g * 2 * CA * 8:(g + 1) * 2 * CA * 8],
                        num_idxs=nA, num_idxs_reg=nA, elem_size=P,
                        single_packet=False, queue_num=(2 * g) % 4)

                for g in range(GDELAY):
                    emit_A(g)
                for g in range(NPAIR):
                    G = Gs.pop(g)
                    nc.gpsimd.dma_gather(
                        out_ap=G[:, 2 * CA:2 * CW, :], in_ap=tabB,
                        idxs_ap=ixB_t[:, g * 2 * CB * 8:(g + 1) * 2 * CB * 8],
                        num_idxs=nB, num_idxs_reg=nB, elem_size=P,
                        single_packet=False, queue_num=(2 * g + 1) % 4)
                    if g + GDELAY < NPAIR:
                        emit_A(g + GDELAY)
                    M2 = mp.tile([P, 2 * CW, P], f16, tag="M")
                    nc.vector.tensor_tensor(
                        out=M2[:], in0=bmid(iota_t[:], 2 * CW),
                        in1=blast(dl_t[:, 2 * g * CW:(2 * g + 2) * CW], P),
                        op=Alu.is_equal)
                    for h in (0, 1):
                        w = 2 * g + h
                        wb = slice(w * P, (w + 1) * P)
                        M = M2[:, h * CW:(h + 1) * CW, :]
                        ps = psA.tile([P, P], fp, tag="ps")
                        for c in range(CW):
                            Gsl = (G[:, h * CA + c, :] if c < CA
                                   else G[:, 2 * CA + h * CB + (c - CA), :])
                            nc.tensor.matmul(out=ps[:], lhsT=Gsl,
                                             rhs=M[:, c, :],
                                             start=(c == 0), stop=(c == CW - 1))
                        if tx_prev2 is None:
                            nc.vector.tensor_tensor(
                                out=tx_new[:, wb], in0=ps[:],
                                in1=d1_t[:, wb], op=Alu.mult)
                        else:
                            tm = tmpp.tile([P, P], fp, tag="tm")
                            nc.vector.tensor_tensor(
                                out=tm[:], in0=ps[:], in1=d2_t[:, wb],
                                op=Alu.mult)
                            nc.vector.tensor_tensor(
                                out=tx_new[:, wb], in0=tm[:],
                                in1=tx_prev2[:, wb], op=Alu.subtract)
                        if want_slice:
                            store_win(tx_new, w, slices)
                    # ---- per-pair: W-matmul chunk, acc update, finales ----
                    ch = slice(2 * g * P, (2 * g + 2) * P)
                    pw = psW.tile([P, 2 * P], fp, tag="pw")
                    nc.tensor.matmul(out=pw[:fo, :], lhsT=Wt[:, k, :fo],
                                     rhs=tx_new[:, ch], start=True, stop=True)
                    nc.vector.tensor_tensor(out=acc[:fo, ch],
                                            in0=acc[:fo, ch],
                                            in1=pw[:fo, :], op=Alu.add)
                    if produce_h is not None:
                        hT, slicesH, tabsH = produce_h
                        nc.scalar.activation(out=hT[:, ch], in_=acc[:, ch],
                                             func=Act.Relu)
                        for h in (0, 1):
                            store_win(hT, 2 * g + h, slicesH)
                        if g == WSPLIT // 2 - 1:
                            tabsH[0] = ag_half(slicesH[0], 0)
                    elif produce_out:
                        for h in (0, 1):
                            w = 2 * g + h
                            wb = slice(w * P, (w + 1) * P)
                            pst = psT.tile([P, P], f16, tag="pst")
                            nc.tensor.transpose(out=pst[:, :CLS],
                                                in_=acc[:CLS, wb],
                                                identity=ident_t[:CLS, :CLS])
                            nm = stp.tile([P, 1], fp, tag="nm")
                            nc.vector.tensor_reduce(
                                out=nm[:], in_=pst[:, :CLS], op=Alu.max,
                                axis=mybir.AxisListType.X, negate=True)
                            ex = stp.tile([P, CLS], fp, tag="ex")
                            ssum = stp.tile([P, 1], fp, tag="ssum")
                            nc.scalar.activation(out=ex[:], in_=pst[:, :CLS],
                                                 func=Act.Exp, bias=nm[:, 0:1],
                                                 accum_out=ssum[:, 0:1])
                            lse = stp.tile([P, 1], fp, tag="lse")
                            nc.scalar.activation(out=lse[:], in_=ssum[:],
                                                 func=Act.Ln)
                            res = stp.tile([P, CLS], fp, tag="res")
                            nc.vector.tensor_scalar(
                                out=res[:], in0=pst[:, :CLS],
                                scalar1=nm[:, 0:1], scalar2=lse[:, 0:1],
                                op0=Alu.add, op1=Alu.subtract)
                            nc.scalar.dma_start(
                                out=out_d[w * P:(w + 1) * P, :], in_=res[:])
                    if want_slice and g == WSPLIT // 2 - 1:
                        tabs[0] = ag_half(slices[0], 0)
                if want_slice:
                    tabs[1] = ag_half(slices[1], 1)
                return tx_new, tabs

            for l, (Wt, b_t, fo) in enumerate(
                    ((w0_t, b0_t, HID), (w1_t, b1_t, HID), (w2_t, b2_t, CLS))):
                if l * 10 >= stage:
                    break
                last = l == 2
                acc = accp.tile([P, SL], f16, tag="acc")
                # ---- k=0 term: acc = W[0].T @ tx0 + b ----
                for g in range(NPAIR):
                    ch = slice(2 * g * P, (2 * g + 2) * P)
                    pw = psW.tile([P, 2 * P], fp, tag="pw")
                    nc.tensor.matmul(out=pw[:fo, :], lhsT=Wt[:, 0, :fo],
                                     rhs=tx0[:, ch], start=True, stop=True)
                    nc.vector.tensor_scalar(
                        out=acc[:fo, ch], in0=pw[:fo, :],
                        scalar1=b_t[:fo, 0:1], scalar2=None, op0=Alu.add)
                # ---- k=1..3 ----
                if stage < l * 10 + 2:
                    break
                tx1, t1 = spmm(tabA_in, tabB_in, None, Wt, b_t, fo, acc,
                               1, True)
                if stage < l * 10 + 4:
                    break
                tx2, t2 = spmm(t1[0][:, :], t1[1][:, :], tx0,
                               Wt, b_t, fo, acc, 2, True)
                if stage < l * 10 + 6:
                    break
                if not last:
                    hT = txp.tile([P, SL], f16, tag="tx")
                    slicesH = [dramp.tile([HROWSA if hh == 0 else HROWSB, F],
                                          f16, tag="sl", name=f"slh{hh}")
                               for hh in (0, 1)]
                    tabsH = [None, None]
                    spmm(t2[0][:, :], t2[1][:, :], tx1,
                         Wt, b_t, fo, acc, 3, False,
                         produce_h=(hT, slicesH, tabsH))
                    tabsH[1] = ag_half(slicesH[1], 1)
                    tx0 = hT
                    tabA_in, tabB_in = tabsH[0][:, :], tabsH[1][:, :]
                else:
                    spmm(t2[0][:, :], t2[1][:, :], tx1,
                         Wt, b_t, fo, acc, 3, False, produce_out=True)

    nc.compile()
    return nc


_CACHE = {}


def _get_nc(CA, CB, stage=99):
    key = (CA, CB, stage)
    if key not in _CACHE:
        _CACHE[key] = _build(CA, CB, stage)
    return _CACHE[key]


def _run(x, edge_src, edge_dst, W0, b0, W1, b1, W2, b2,
         trace=False, trace_cores=None, stage=99):
    from concourse import bass_utils

    n = x.shape[0]
    pre = _preprocess(edge_src, edge_dst, n)
    perm, CA, CB = pre["perm"], pre["CA"], pre["CB"]
    dinv = pre["dinv_new"]  # by new node id

    x = np.asarray(x, np.float32)
    x_pad = np.zeros((NPAD, F), np.float32)
    x_pad[perm] = x

    # prescaled table in half-major row layout
    rows = _table_row(np.arange(NPAD))
    xfull = np.zeros((NPAD, F), np.float16)
    xfull[rows] = (x_pad * dinv[:, None]).astype(np.float16)

    w0t = np.transpose(np.asarray(W0, np.float32), (1, 0, 2)).astype(np.float16)
    w1t = np.transpose(np.asarray(W1, np.float32), (1, 0, 2)).astype(np.float16)
    w2t = np.transpose(np.asarray(W2, np.float32), (1, 0, 2)).astype(np.float16)
    w0t, w1t, w2t = (np.ascontiguousarray(a) for a in (w0t, w1t, w2t))
    iota = np.ascontiguousarray(
        np.broadcast_to(np.arange(P, dtype=np.float16), (P, P)))
    ident = np.eye(P, dtype=np.float16)

    in_maps = []
    for c in range(CORES):
        rows_c = slice(c * SL, (c + 1) * SL)
        dinv_c = dinv[rows_c]  # [6400] by local node id (w*128+p)
        in_maps.append(dict(
            xT=np.ascontiguousarray(x_pad[rows_c].T).astype(np.float16),
            xfull=xfull,
            ixA=pre["idxA"][c], ixB=pre["idxB"][c],
            dl=pre["dl"][c].astype(np.float16),
            d1=np.ascontiguousarray(
                np.broadcast_to(-dinv_c, (P, SL))).astype(np.float16),
            d2=np.ascontiguousarray(
                np.broadcast_to(-2.0 * dinv_c, (P, SL))).astype(np.float16),
            dinvP=np.ascontiguousarray(
                dinv_c.reshape(NW, P).T).astype(np.float32),
            w0t=w0t, w1t=w1t, w2t=w2t,
            b0=np.asarray(b0, np.float32).reshape(HID, 1),
            b1=np.asarray(b1, np.float32).reshape(HID, 1),
            b2=np.asarray(b2, np.float32).reshape(CLS, 1),
            iota=iota, ident=ident,
        ))

    nc = _get_nc(CA, CB, stage)
    kw = {}
    if trace:
        kw = dict(trace=True,
                  trace_cores=trace_cores if trace_cores is not None else [0])
    res = bass_utils.run_bass_kernel_spmd(nc, in_maps,
                                          core_ids=list(range(CORES)), **kw)

    full = np.concatenate([res.results[c]["out"] for c in range(CORES)],
                          axis=0)
    out = full[perm]
    return out.astype(np.float32), res


def kernel(x, edge_src, edge_dst, W0, b0, W1, b1, W2, b2):
    out, _ = _run(x, edge_src, edge_dst, W0, b0, W1, b1, W2, b2)
    return out
